# revision 30
# baseline (speedup 1.0000x reference)
"""DecoderRNN single-step (embed+ReLU -> GRU cell -> vocab projection -> log_softmax)
as a tensor-parallel Bass/Tile kernel on 8 TRN2 NeuronCores.

Strategy:
  - GRU: replicated — every core computes the full GRU cell on the TensorEngine
    (25MB of f32 GRU weights become 12.6MB as bf16; replicating them avoids an
    h_new AllGather, which in this environment costs far more than the extra DMA).
  - Output projection: vocab padded to 53248 = 8*6656 and sharded contiguously;
    each core computes its 6656 logits on the TensorEngine with host-side
    pre-transposed bf16 weights, then exp-sums them.
  - log_softmax: the per-core exp-sums (one scalar each) are AllGather'd — the
    only collective — and every core computes the global logsumexp locally and
    writes its logp shard.

Contraction layout: the hidden dim is consumed in 8 chunks of 128 with the
permutation h = 8p + j (partition p, chunk j), so vectors in "column layout"
[128, 8] are plain row-major reshapes of the length-1024 vector. The GRU gate
ROWS are interleaved the same way (gate row 8m + j at psum[m, j]) so h_new is
produced directly in the layout the projection consumes — no data movement
between the GRU and the projection.

Weights are bf16 (host-cast); activations/accumulation stay f32. The embedding
table is replicated; the row gather is one indirect DMA with per-partition
offsets token*128 + p into a [V*128, 8] view of the table.
"""

import numpy as np
import ml_dtypes

import concourse.bass as bass
import concourse.tile as tile
from concourse import bacc, mybir
from concourse.bass_utils import run_bass_kernel_spmd

HIDDEN = 1024
VOCAB = 50257
N_CORES = 8
VPAD = 53248            # 8 * 6656, multiple of 8*128
VSH = VPAD // N_CORES   # 6656 vocab rows per core
NT = VSH // 128         # 52 vocab tiles per core
NDMA = 4                # projection weight stream quarters (DMA granularity)
DW = VSH // NDMA        # 1664 columns per DMA chunk
DT = NT // NDMA         # 13 tiles per DMA quarter
NG = 13                 # psum/evacuation groups (small so the last one is quick)
GT = NT // NG           # 4 tiles per group
NJ = HIDDEN // 128      # 8 contraction chunks
PAD_BIAS = -1e30
WOUT_SCALE = 64.0       # fp8 projection weights stored as w*64; h fed as h/64
GRU_SCALE = 64.0        # fp8 GRU weights stored as w*64; x,h fed as v/64

F32 = mybir.dt.float32
BF16 = mybir.dt.bfloat16
FP8 = mybir.dt.float8e4
I32 = mybir.dt.int32
BF16_NP = ml_dtypes.bfloat16

_CACHE = {}


def _build_nc():
    nc = bacc.Bacc(
        "TRN2",
        target_bir_lowering=False,
        debug=False,
        num_devices=N_CORES,
    )
    # ---- kernel I/O ----
    emb_d = nc.dram_tensor("emb_v", [VOCAB * 128, 8], BF16, kind="ExternalInput")
    token_d = nc.dram_tensor("token32", [1, 1], I32, kind="ExternalInput")
    iota_d = nc.dram_tensor("iota128", [128, 1], F32, kind="ExternalInput")
    hidcol_d = nc.dram_tensor("hid_col", [128, 8], F32, kind="ExternalInput")
    gbias_d = nc.dram_tensor("gru_bias", [128, 4 * 8], F32, kind="ExternalInput")
    # full GRU weights, [gate][p][j][k][m] with row = 8m+j, col = 8p+k.
    # W_ih multiplies the tiny embedding vector -> fp8 (stored as w*GRU_SCALE,
    # compensated by x/GRU_SCALE); W_hh multiplies h (|h|~1, dominates h_new
    # accuracy) -> bf16.
    wih_d = nc.dram_tensor("w_ih_p", [3, 128, NJ, NJ, 128], FP8, kind="ExternalInput")
    whh_d = nc.dram_tensor("w_hh_p", [3, 128, NJ, NJ, 128], BF16, kind="ExternalInput")
    wout_d = nc.dram_tensor("w_out_p", [NJ, 128, VSH], FP8, kind="ExternalInput")
    bout_d = nc.dram_tensor("b_out_col", [128, NT], F32, kind="ExternalInput")
    logp_d = nc.dram_tensor("logp", [128, NT], F32, kind="ExternalOutput")
    hnew_d = nc.dram_tensor("h_new", [128, NJ], F32, kind="ExternalOutput")

    AF = mybir.ActivationFunctionType
    OP = mybir.AluOpType

    with tile.TileContext(nc) as tc:
        with (
            tc.tile_pool(name="small", bufs=1) as small,
            tc.tile_pool(name="big", bufs=1) as big,
            tc.tile_pool(name="psump", bufs=1, space="PSUM") as psump,
            tc.tile_pool(name="dram", bufs=1, space="DRAM") as dram,
        ):
            # ---- small input loads ----
            tok_i = small.tile([128, 1], I32)
            nc.gpsimd.dma_start(out=tok_i[:], in_=token_d.ap().to_broadcast([128, 1]))
            iota_sb = small.tile([128, 1], F32)
            nc.scalar.dma_start(out=iota_sb[:], in_=iota_d.ap())
            hidcol_f = small.tile([128, NJ], F32)
            nc.scalar.dma_start(out=hidcol_f[:], in_=hidcol_d.ap())
            gb = small.tile([128, 4 * 8], F32)
            nc.scalar.dma_start(out=gb[:], in_=gbias_d.ap())
            bout_sb = small.tile([128, NT], F32)
            nc.scalar.dma_start(out=bout_sb[:], in_=bout_d.ap())
            ones_sb = small.tile([128, 1], F32)
            nc.vector.memset(ones_sb[:], 1.0)
            ones_row = small.tile([1, 128], F32)
            nc.vector.memset(ones_row[:], 1.0)
            zero8 = small.tile([1, 8], F32)
            nc.vector.memset(zero8[:], 0.0)

            # first ACT op is an Exp so walrus loads exp_and_others (which also
            # serves every Tanh/Identity/Copy below) once, early
            dum0 = small.tile([1, 1], F32)
            nc.vector.memset(dum0[:], 1.0)
            dum1 = small.tile([1, 1], F32)
            nc.scalar.activation(dum1[:], dum0[:], AF.Exp)

            # ---- early dummy AllGather: absorb first-collective setup cost
            # while the weight stream runs ----
            cc0_in = dram.tile([1, 8], F32)
            cc0_out = dram.tile([N_CORES, 8], F32, addr_space="Shared")
            nc.scalar.dma_start(out=cc0_in[:], in_=zero8[:])
            nc.gpsimd.collective_compute(
                "AllGather",
                OP.bypass,
                replica_groups=[list(range(N_CORES))],
                ins=[cc0_in[:].opt()],
                outs=[cc0_out[:].opt()],
            )

            # ---- embedding row gather (x = emb[token] in column layout) ----
            tok_f = small.tile([128, 1], F32)
            nc.vector.tensor_copy(tok_f[:], tok_i[:])
            offs_f = small.tile([128, 1], F32)
            nc.vector.tensor_scalar(
                offs_f[:], tok_f[:], 128.0, iota_sb[:], OP.mult, OP.add
            )
            offs_i = small.tile([128, 1], I32)
            nc.vector.tensor_copy(offs_i[:], offs_f[:])
            x_raw = small.tile([128, NJ], BF16)
            nc.gpsimd.indirect_dma_start(
                out=x_raw[:],
                out_offset=None,
                in_=emb_d.ap(),
                in_offset=bass.IndirectOffsetOnAxis(ap=offs_i[:], axis=0),
            )
            # x = relu(row)/GRU_SCALE, fused (max then mult), bf16 out
            x_bf = small.tile([128, NJ], BF16)
            nc.vector.tensor_scalar(
                x_bf[:], x_raw[:], 0.0, 1.0 / GRU_SCALE, OP.max, OP.mult
            )
            h_bf = small.tile([128, NJ], BF16)
            nc.vector.tensor_copy(h_bf[:], hidcol_f[:])

            # ---- GRU weight loads (3 slots; 2.1MB per gate-matrix) ----
            # order: ih_r, hh_r, ih_z, hh_z, ih_n, hh_n
            gru_w = []
            for g in range(3):
                for mat_d, wdt in ((wih_d, FP8), (whh_d, BF16)):
                    t = big.tile([128, NJ, NJ, 128], wdt, tag="gru", bufs=3)
                    nc.sync.dma_start(out=t[:], in_=mat_d.ap()[g])
                    gru_w.append(t)

            # ---- output projection weight stream, in two halves so the PE can
            # start the first half as soon as its 8 chunks arrive ----
            w_sb = [[None] * NJ for _ in range(NDMA)]
            for g in range(NDMA):
                for j in range(NJ):
                    t = big.tile([128, DW], FP8, tag=f"w{g}_{j}")
                    eng = nc.sync if j % 2 == 0 else nc.gpsimd
                    eng.dma_start(
                        out=t[:], in_=wout_d.ap()[j][:, g * DW : (g + 1) * DW]
                    )
                    w_sb[g][j] = t

            # ---- GRU gate matvecs on PE (full 1024 rows per gate) ----
            # psum[m, jc] = gate row 8m+jc
            ps_r = psump.tile([128, NJ], F32, tag="ps_r")
            ps_z = psump.tile([128, NJ], F32, tag="ps_z")
            ps_in = psump.tile([128, NJ], F32, tag="ps_in")
            ps_hn = psump.tile([128, NJ], F32, tag="ps_hn")
            for ps, blocks in (
                (ps_r, ((gru_w[0], x_bf), (gru_w[1], h_bf))),
                (ps_z, ((gru_w[2], x_bf), (gru_w[3], h_bf))),
                (ps_in, ((gru_w[4], x_bf),)),
                (ps_hn, ((gru_w[5], h_bf),)),
            ):
                n_mm = len(blocks) * NJ
                for jc in range(NJ):
                    k = 0
                    for w, rhs in blocks:
                        for kc in range(NJ):
                            nc.tensor.matmul(
                                out=ps[:, jc : jc + 1],
                                lhsT=w[:, jc, kc, :],
                                rhs=rhs[:, kc : kc + 1],
                                start=(k == 0),
                                stop=(k == n_mm - 1),
                            )
                            k += 1

            # ---- GRU elementwise ([128, 8] tiles; biases vary per element so
            # they're added on DVE; sigmoid(v) = 0.5*tanh(0.5*v)+0.5 keeps all
            # ACT ops inside the exp_and_others table set) ----
            # gru_bias cols: [0:8]=(b_ih+b_hh)_r, [8:16]=(b_ih+b_hh)_z,
            #                [16:24]=b_ih_n, [24:32]=b_hh_n  (all interleaved)
            pre_r = small.tile([128, NJ], F32)
            nc.vector.tensor_tensor(pre_r[:], ps_r[:], gb[:, 0:8], op=OP.add)
            r_t = small.tile([128, NJ], F32)
            nc.scalar.activation(r_t[:], pre_r[:], AF.Tanh, scale=0.5)
            r_sb = small.tile([128, NJ], F32)
            nc.vector.tensor_scalar(r_sb[:], r_t[:], 0.5, 0.5, OP.mult, OP.add)

            pre_z = small.tile([128, NJ], F32)
            nc.vector.tensor_tensor(pre_z[:], ps_z[:], gb[:, 8:16], op=OP.add)
            z_t = small.tile([128, NJ], F32)
            nc.scalar.activation(z_t[:], pre_z[:], AF.Tanh, scale=0.5)
            z_sb = small.tile([128, NJ], F32)
            nc.vector.tensor_scalar(z_sb[:], z_t[:], 0.5, 0.5, OP.mult, OP.add)

            inb = small.tile([128, NJ], F32)
            nc.vector.tensor_tensor(inb[:], ps_in[:], gb[:, 16:24], op=OP.add)
            hnb = small.tile([128, NJ], F32)
            nc.vector.tensor_tensor(hnb[:], ps_hn[:], gb[:, 24:32], op=OP.add)
            rhn = small.tile([128, NJ], F32)
            nc.vector.tensor_tensor(rhn[:], r_sb[:], hnb[:], op=OP.mult)
            pre_n = small.tile([128, NJ], F32)
            nc.vector.tensor_tensor(pre_n[:], rhn[:], inb[:], op=OP.add)
            n_sb = small.tile([128, NJ], F32)
            nc.scalar.activation(n_sb[:], pre_n[:], AF.Tanh)
            d_sb = small.tile([128, NJ], F32)
            nc.vector.tensor_tensor(d_sb[:], hidcol_f[:], n_sb[:], op=OP.subtract)
            zd = small.tile([128, NJ], F32)
            nc.vector.tensor_tensor(zd[:], z_sb[:], d_sb[:], op=OP.mult)
            hnew_sb = small.tile([128, NJ], F32)
            nc.vector.tensor_tensor(hnew_sb[:], n_sb[:], zd[:], op=OP.add)

            nc.scalar.dma_start(out=hnew_d.ap(), in_=hnew_sb[:])
            # h/WOUT_SCALE in bf16: compensates the *WOUT_SCALE stored in the
            # fp8 projection weights (keeps them in e4m3's normal range)
            hs_f = small.tile([128, NJ], F32)
            nc.vector.tensor_scalar_mul(hs_f[:], hnew_sb[:], 1.0 / WOUT_SCALE)
            hcol_bf = small.tile([128, NJ], BF16)
            nc.vector.tensor_copy(hcol_bf[:], hs_f[:])

            # ---- output projection: logits + exp-sums, group by group ----
            logits_sb = small.tile([128, NT], F32)
            sums = small.tile([128, NG], F32)
            for g in range(NG):
                ps = psump.tile([128, GT], F32, tag="lps", bufs=2)
                for t in range(GT):
                    tg = g * GT + t          # global v-tile index
                    dg, td = divmod(tg, DT)  # DMA half + tile within it
                    for j in range(NJ):
                        nc.tensor.matmul(
                            out=ps[:, t : t + 1],
                            lhsT=w_sb[dg][j][:, td * 128 : (td + 1) * 128],
                            rhs=hcol_bf[:, j : j + 1],
                            start=(j == 0),
                            stop=(j == NJ - 1),
                        )
                gsl = slice(g * GT, (g + 1) * GT)
                nc.vector.tensor_tensor(
                    logits_sb[:, gsl], ps[:], bout_sb[:, gsl], op=OP.add
                )
                esc = small.tile([128, GT], F32, tag="esc", bufs=2)
                nc.scalar.activation(
                    esc[:], logits_sb[:, gsl], AF.Exp, accum_out=sums[:, g : g + 1]
                )

            # ---- global logsumexp via AllGather of per-core exp sums ----
            stot = small.tile([128, 1], F32)
            nc.vector.tensor_reduce(stot[:], sums[:], axis=mybir.AxisListType.X, op=OP.add)
            ps_s = psump.tile([1, 1], F32, tag="ps_s")
            nc.tensor.matmul(out=ps_s[:], lhsT=ones_sb[:], rhs=stot[:], start=True, stop=True)
            s_sb = small.tile([1, 1], F32)
            nc.scalar.copy(s_sb[:], ps_s[:])
            # replicate the scalar across 8 lanes so both collective bounce
            # buffers are single-descriptor contiguous transfers; the 8x
            # over-count is folded into the Ln scale below.
            s8 = small.tile([1, 8], F32)
            nc.vector.tensor_scalar(s8[:], zero8[:], s_sb[0:1, 0:1], None, OP.add)
            cc2_in = dram.tile([1, 8], F32)
            cc2_out = dram.tile([N_CORES, 8], F32, addr_space="Shared")
            nc.scalar.dma_start(out=cc2_in[:], in_=s8[:])
            # load the natural_log table set while AG is in flight
            duml = small.tile([1, 1], F32)
            nc.scalar.activation(duml[:], dum1[:], AF.Ln)
            nc.gpsimd.collective_compute(
                "AllGather",
                OP.bypass,
                replica_groups=[list(range(N_CORES))],
                ins=[cc2_in[:].opt()],
                outs=[cc2_out[:].opt()],
            )
            s64 = small.tile([1, 64], F32)
            nc.scalar.dma_start(
                out=s64[:], in_=cc2_out[:].rearrange("(o a) b -> o (a b)", o=1)
            )
            stot2 = small.tile([1, 1], F32)
            nc.vector.tensor_reduce(
                stot2[:], s64[:], axis=mybir.AxisListType.X, op=OP.add
            )
            # lse = ln(sum_c s_c) = ln(0.125 * sum(s64))
            lse1 = small.tile([1, 1], F32)
            nc.scalar.activation(lse1[:], stot2[:], AF.Ln, scale=0.125)
            # broadcast lse to all partitions via PE (ones_row.T @ lse1)
            ps_l = psump.tile([128, 1], F32, tag="ps_l")
            nc.tensor.matmul(out=ps_l[:], lhsT=ones_row[:], rhs=lse1[:], start=True, stop=True)
            lse_b = small.tile([128, 1], F32)
            nc.scalar.copy(lse_b[:], ps_l[:])

            logp_sb = small.tile([128, NT], F32)
            nc.vector.tensor_scalar(
                logp_sb[:], logits_sb[:], lse_b[:], None, OP.subtract
            )
            nc.scalar.dma_start(out=logp_d.ap(), in_=logp_sb[:])

    nc.compile()
    return nc


def get_nc():
    if "nc" not in _CACHE:
        _CACHE["nc"] = _build_nc()
    return _CACHE["nc"]


def prepare_in_maps(token, hidden, emb, w_ih, w_hh, b_ih, b_hh, w_out, b_out):
    token = np.asarray(token).reshape(-1)
    hidden = np.asarray(hidden, dtype=np.float32).reshape(HIDDEN)
    emb = np.asarray(emb, dtype=np.float32)
    w_ih = np.asarray(w_ih, dtype=np.float32)
    w_hh = np.asarray(w_hh, dtype=np.float32)
    b_ih = np.asarray(b_ih, dtype=np.float32)
    b_hh = np.asarray(b_hh, dtype=np.float32)
    w_out = np.asarray(w_out, dtype=np.float32)
    b_out = np.asarray(b_out, dtype=np.float32)

    emb_v = np.ascontiguousarray(emb).reshape(VOCAB * 128, 8).astype(BF16_NP)
    token32 = np.array([[int(token[0])]], dtype=np.int32)
    iota128 = np.arange(128, dtype=np.float32).reshape(128, 1)
    hid_col = np.ascontiguousarray(hidden.reshape(128, NJ))

    # GRU weights: [3h, h] -> [3, p, j, k, m]; element = W[g*1024+8m+j, 8p+k]
    def prep_gru(w, scale, np_dt):
        a = w.reshape(3, 128, NJ, 128, NJ)  # (g, m, j, p, k)
        return (np.ascontiguousarray(a.transpose(0, 3, 2, 4, 1)) * scale).astype(np_dt)

    wih_p = prep_gru(w_ih, GRU_SCALE, mybir.dt.np(FP8))
    whh_p = prep_gru(w_hh, 1.0, BF16_NP)

    # biases in the interleaved [p, j] layout (value for gate row 8p+j)
    def il(v):
        return v.reshape(128, NJ)

    # full bias here: the 0.5 of the tanh-based sigmoid is applied by the ACT
    # scale AFTER the DVE bias-add (tanh(0.5*(psum + b)))
    b_r = il(b_ih[0:HIDDEN] + b_hh[0:HIDDEN])
    b_z = il(b_ih[HIDDEN : 2 * HIDDEN] + b_hh[HIDDEN : 2 * HIDDEN])
    b_in = il(b_ih[2 * HIDDEN :])
    b_hn = il(b_hh[2 * HIDDEN :])
    gru_bias = np.ascontiguousarray(
        np.concatenate([b_r, b_z, b_in, b_hn], axis=1)
    )

    # output projection: pad vocab, [V, h] -> per core [j, p, v] (h-col = 8p+j)
    w_out_pad = np.zeros((VPAD, HIDDEN), dtype=np.float32)
    w_out_pad[:VOCAB] = w_out
    wp = w_out_pad.reshape(N_CORES, VSH, 128, NJ)  # (c, v, p, j)
    b_out_pad = np.full(VPAD, PAD_BIAS, dtype=np.float32)
    b_out_pad[:VOCAB] = b_out
    bp = b_out_pad.reshape(N_CORES, NT, 128)

    in_maps = []
    for c in range(N_CORES):
        in_maps.append(
            {
                "emb_v": emb_v,
                "token32": token32,
                "iota128": iota128,
                "hid_col": hid_col,
                "gru_bias": gru_bias,
                "w_ih_p": wih_p,
                "w_hh_p": whh_p,
                "w_out_p": (
                    np.ascontiguousarray(wp[c].transpose(2, 1, 0)) * WOUT_SCALE
                ).astype(mybir.dt.np(FP8)),
                "b_out_col": np.ascontiguousarray(bp[c].T),
            }
        )
    return in_maps


def assemble_outputs(results):
    logp_pad = np.empty(VPAD, dtype=np.float32)
    for c in range(N_CORES):
        lp = results[c]["logp"]  # [128, NT], v_local = t*128 + p
        logp_pad[c * VSH : (c + 1) * VSH] = lp.T.reshape(VSH)
    # h_new is replicated; [128, 8] row-major == natural order (h = 8p+j)
    hnew = results[0]["h_new"].reshape(HIDDEN).astype(np.float32)
    return logp_pad[:VOCAB].reshape(1, VOCAB), hnew.reshape(1, 1, HIDDEN)


def run(inputs, **spmd_kwargs):
    nc = get_nc()
    in_maps = prepare_in_maps(**inputs)
    res = run_bass_kernel_spmd(nc, in_maps, core_ids=list(range(N_CORES)), **spmd_kwargs)
    return assemble_outputs(res.results), res


def kernel(**inputs):
    outputs, _ = run(inputs)
    return outputs


# revision 32
# speedup vs baseline: 1.2103x; 1.2103x over previous
"""DecoderRNN single-step (embed+ReLU -> GRU cell -> vocab projection -> log_softmax)
as a tensor-parallel Bass/Tile kernel on 8 TRN2 NeuronCores.

Strategy:
  - GRU: replicated — every core computes the full GRU cell on the TensorEngine;
    replicating the (quantized) GRU weights avoids an h_new AllGather, which in
    this environment costs far more than the extra DMA. W_ih is fp8 (x is tiny,
    error negligible), W_hh is bf16 (dominates h_new accuracy).
  - Output projection: vocab padded to 53248 = 8*6656 and sharded contiguously;
    each core computes its 6656 logits on the TensorEngine with host-side
    pre-transposed fp8 weights (stored as w*64, compensated by h/64 in bf16;
    f32 PSUM accumulation), then exp-sums them.
  - log_softmax: the per-core exp-sums (one scalar each) are AllGather'd — the
    only collective — and every core computes the global logsumexp locally and
    writes its logp shard.

Contraction layout: the hidden dim is consumed in 8 chunks of 128 with the
permutation h = 8p + j (partition p, chunk j), so vectors in "column layout"
[128, 8] are plain row-major reshapes of the length-1024 vector. The GRU gate
ROWS are interleaved the same way (gate row 8m + j at psum[m, j]) so h_new is
produced directly in the layout the projection consumes — no data movement
between the GRU and the projection.

Weights are bf16 (host-cast); activations/accumulation stay f32. The embedding
table is replicated; the row gather is one indirect DMA with per-partition
offsets token*128 + p into a [V*128, 8] view of the table.
"""

import numpy as np
import ml_dtypes

import concourse.bass as bass
import concourse.tile as tile
from concourse import bacc, mybir
from concourse.bass_utils import run_bass_kernel_spmd

HIDDEN = 1024
VOCAB = 50257
N_CORES = 8
VPAD = 53248            # 8 * 6656, multiple of 8*128
VSH = VPAD // N_CORES   # 6656 vocab rows per core
NT = VSH // 128         # 52 vocab tiles per core
NDMA = 4                # projection weight stream quarters (DMA granularity)
DW = VSH // NDMA        # 1664 columns per DMA chunk
DT = NT // NDMA         # 13 tiles per DMA quarter
NG = 13                 # psum/evacuation groups (small so the last one is quick)
GT = NT // NG           # 4 tiles per group
NJ = HIDDEN // 128      # 8 contraction chunks
PAD_BIAS = -1e30
WOUT_SCALE = 64.0       # fp8 projection weights stored as w*64; h fed as h/64
GRU_SCALE = 64.0        # fp8 GRU weights stored as w*64; x,h fed as v/64

F32 = mybir.dt.float32
BF16 = mybir.dt.bfloat16
FP8 = mybir.dt.float8e4
I32 = mybir.dt.int32
BF16_NP = ml_dtypes.bfloat16

_CACHE = {}


def _build_nc():
    nc = bacc.Bacc(
        "TRN2",
        target_bir_lowering=False,
        debug=False,
        num_devices=N_CORES,
    )
    # ---- kernel I/O ----
    emb_d = nc.dram_tensor("emb_v", [VOCAB * 128, 8], BF16, kind="ExternalInput")
    token_d = nc.dram_tensor("token32", [1, 1], I32, kind="ExternalInput")
    iota_d = nc.dram_tensor("iota128", [128, 1], F32, kind="ExternalInput")
    hidcol_d = nc.dram_tensor("hid_col", [128, 8], F32, kind="ExternalInput")
    gbias_d = nc.dram_tensor("gru_bias", [128, 4 * 8], F32, kind="ExternalInput")
    # full GRU weights, [gate][p][j][k][m] with row = 8m+j, col = 8p+k.
    # W_ih multiplies the tiny embedding vector -> fp8 (stored as w*GRU_SCALE,
    # compensated by x/GRU_SCALE); W_hh multiplies h (|h|~1, dominates h_new
    # accuracy) -> bf16.
    wih_d = nc.dram_tensor("w_ih_p", [3, 128, NJ, NJ, 128], FP8, kind="ExternalInput")
    whhr_d = nc.dram_tensor("w_hh_r", [1, 128, NJ, NJ, 128], FP8, kind="ExternalInput")
    whh_d = nc.dram_tensor("w_hh_p", [2, 128, NJ, NJ, 128], BF16, kind="ExternalInput")
    wout_d = nc.dram_tensor("w_out_p", [NJ, 128, VSH], FP8, kind="ExternalInput")
    bout_d = nc.dram_tensor("b_out_col", [128, NT], F32, kind="ExternalInput")
    logp_d = nc.dram_tensor("logp", [128, NT], F32, kind="ExternalOutput")
    hnew_d = nc.dram_tensor("h_new", [128, NJ], F32, kind="ExternalOutput")

    AF = mybir.ActivationFunctionType
    OP = mybir.AluOpType

    with tile.TileContext(nc) as tc:
        with (
            tc.tile_pool(name="small", bufs=1) as small,
            tc.tile_pool(name="big", bufs=1) as big,
            tc.tile_pool(name="psump", bufs=1, space="PSUM") as psump,
            tc.tile_pool(name="dram", bufs=1, space="DRAM") as dram,
        ):
            # ---- small input loads ----
            tok_i = small.tile([128, 1], I32)
            nc.gpsimd.dma_start(out=tok_i[:], in_=token_d.ap().to_broadcast([128, 1]))
            iota_sb = small.tile([128, 1], F32)
            nc.scalar.dma_start(out=iota_sb[:], in_=iota_d.ap())
            hidcol_f = small.tile([128, NJ], F32)
            nc.scalar.dma_start(out=hidcol_f[:], in_=hidcol_d.ap())
            gb = small.tile([128, 4 * 8], F32)
            nc.scalar.dma_start(out=gb[:], in_=gbias_d.ap())
            bout_sb = small.tile([128, NT], F32)
            nc.scalar.dma_start(out=bout_sb[:], in_=bout_d.ap())
            ones_sb = small.tile([128, 1], F32)
            nc.vector.memset(ones_sb[:], 1.0)
            ones_row = small.tile([1, 128], F32)
            nc.vector.memset(ones_row[:], 1.0)
            zero8 = small.tile([1, 8], F32)
            nc.vector.memset(zero8[:], 0.0)

            # first ACT op is an Exp so walrus loads exp_and_others (which also
            # serves every Tanh/Identity/Copy below) once, early
            dum0 = small.tile([1, 1], F32)
            nc.vector.memset(dum0[:], 1.0)
            dum1 = small.tile([1, 1], F32)
            nc.scalar.activation(dum1[:], dum0[:], AF.Exp)

            # ---- early dummy AllGather: absorb first-collective setup cost
            # while the weight stream runs ----
            cc0_in = dram.tile([1, 8], F32)
            cc0_out = dram.tile([N_CORES, 8], F32, addr_space="Shared")
            nc.scalar.dma_start(out=cc0_in[:], in_=zero8[:])
            nc.gpsimd.collective_compute(
                "AllGather",
                OP.bypass,
                replica_groups=[list(range(N_CORES))],
                ins=[cc0_in[:].opt()],
                outs=[cc0_out[:].opt()],
            )

            # ---- embedding row gather (x = emb[token] in column layout) ----
            tok_f = small.tile([128, 1], F32)
            nc.vector.tensor_copy(tok_f[:], tok_i[:])
            offs_f = small.tile([128, 1], F32)
            nc.vector.tensor_scalar(
                offs_f[:], tok_f[:], 128.0, iota_sb[:], OP.mult, OP.add
            )
            offs_i = small.tile([128, 1], I32)
            nc.vector.tensor_copy(offs_i[:], offs_f[:])
            x_raw = small.tile([128, NJ], BF16)
            nc.gpsimd.indirect_dma_start(
                out=x_raw[:],
                out_offset=None,
                in_=emb_d.ap(),
                in_offset=bass.IndirectOffsetOnAxis(ap=offs_i[:], axis=0),
            )
            # x = relu(row)/GRU_SCALE, fused (max then mult), bf16 out
            x_bf = small.tile([128, NJ], BF16)
            nc.vector.tensor_scalar(
                x_bf[:], x_raw[:], 0.0, 1.0 / GRU_SCALE, OP.max, OP.mult
            )
            h_bf = small.tile([128, NJ], BF16)
            nc.vector.tensor_copy(h_bf[:], hidcol_f[:])
            # h/GRU_SCALE for the fp8 W_hh_r matmuls
            hs_bf = small.tile([128, NJ], BF16)
            nc.vector.tensor_scalar(
                hs_bf[:], hidcol_f[:], 1.0 / GRU_SCALE, None, OP.mult
            )

            # ---- GRU weight loads (3 slots; 2.1MB per gate-matrix) ----
            # order: ih_r, hh_r, ih_z, hh_z, ih_n, hh_n
            gru_w = []
            srcs = [
                (wih_d, 0, FP8), (whhr_d, 0, FP8),   # r gate
                (wih_d, 1, FP8), (whh_d, 0, BF16),   # z gate
                (wih_d, 2, FP8), (whh_d, 1, BF16),   # n gates
            ]
            for mat_d, gi, wdt in srcs:
                t = big.tile([128, NJ, NJ, 128], wdt, tag="gru", bufs=3)
                nc.sync.dma_start(out=t[:], in_=mat_d.ap()[gi])
                gru_w.append(t)

            # ---- output projection weight stream, in two halves so the PE can
            # start the first half as soon as its 8 chunks arrive ----
            w_sb = [[None] * NJ for _ in range(NDMA)]
            for g in range(NDMA):
                for j in range(NJ):
                    t = big.tile([128, DW], FP8, tag=f"w{g}_{j}")
                    eng = nc.sync if j % 2 == 0 else nc.gpsimd
                    eng.dma_start(
                        out=t[:], in_=wout_d.ap()[j][:, g * DW : (g + 1) * DW]
                    )
                    w_sb[g][j] = t

            # ---- GRU gate matvecs on PE (full 1024 rows per gate) ----
            # psum[m, jc] = gate row 8m+jc
            ps_r = psump.tile([128, NJ], F32, tag="ps_r")
            ps_z = psump.tile([128, NJ], F32, tag="ps_z")
            ps_in = psump.tile([128, NJ], F32, tag="ps_in")
            ps_hn = psump.tile([128, NJ], F32, tag="ps_hn")
            for ps, blocks in (
                (ps_r, ((gru_w[0], x_bf), (gru_w[1], hs_bf))),
                (ps_z, ((gru_w[2], x_bf), (gru_w[3], h_bf))),
                (ps_in, ((gru_w[4], x_bf),)),
                (ps_hn, ((gru_w[5], h_bf),)),
            ):
                n_mm = len(blocks) * NJ
                for jc in range(NJ):
                    k = 0
                    for w, rhs in blocks:
                        for kc in range(NJ):
                            nc.tensor.matmul(
                                out=ps[:, jc : jc + 1],
                                lhsT=w[:, jc, kc, :],
                                rhs=rhs[:, kc : kc + 1],
                                start=(k == 0),
                                stop=(k == n_mm - 1),
                            )
                            k += 1

            # ---- GRU elementwise ([128, 8] tiles; biases vary per element so
            # they're added on DVE; sigmoid(v) = 0.5*tanh(0.5*v)+0.5 keeps all
            # ACT ops inside the exp_and_others table set) ----
            # gru_bias cols: [0:8]=(b_ih+b_hh)_r, [8:16]=(b_ih+b_hh)_z,
            #                [16:24]=b_ih_n, [24:32]=b_hh_n  (all interleaved)
            pre_r = small.tile([128, NJ], F32)
            nc.vector.tensor_tensor(pre_r[:], ps_r[:], gb[:, 0:8], op=OP.add)
            r_t = small.tile([128, NJ], F32)
            nc.scalar.activation(r_t[:], pre_r[:], AF.Tanh, scale=0.5)
            r_sb = small.tile([128, NJ], F32)
            nc.vector.tensor_scalar(r_sb[:], r_t[:], 0.5, 0.5, OP.mult, OP.add)

            pre_z = small.tile([128, NJ], F32)
            nc.vector.tensor_tensor(pre_z[:], ps_z[:], gb[:, 8:16], op=OP.add)
            z_t = small.tile([128, NJ], F32)
            nc.scalar.activation(z_t[:], pre_z[:], AF.Tanh, scale=0.5)
            z_sb = small.tile([128, NJ], F32)
            nc.vector.tensor_scalar(z_sb[:], z_t[:], 0.5, 0.5, OP.mult, OP.add)

            inb = small.tile([128, NJ], F32)
            nc.vector.tensor_tensor(inb[:], ps_in[:], gb[:, 16:24], op=OP.add)
            hnb = small.tile([128, NJ], F32)
            nc.vector.tensor_tensor(hnb[:], ps_hn[:], gb[:, 24:32], op=OP.add)
            rhn = small.tile([128, NJ], F32)
            nc.vector.tensor_tensor(rhn[:], r_sb[:], hnb[:], op=OP.mult)
            pre_n = small.tile([128, NJ], F32)
            nc.vector.tensor_tensor(pre_n[:], rhn[:], inb[:], op=OP.add)
            n_sb = small.tile([128, NJ], F32)
            nc.scalar.activation(n_sb[:], pre_n[:], AF.Tanh)
            d_sb = small.tile([128, NJ], F32)
            nc.vector.tensor_tensor(d_sb[:], hidcol_f[:], n_sb[:], op=OP.subtract)
            zd = small.tile([128, NJ], F32)
            nc.vector.tensor_tensor(zd[:], z_sb[:], d_sb[:], op=OP.mult)
            hnew_sb = small.tile([128, NJ], F32)
            nc.vector.tensor_tensor(hnew_sb[:], n_sb[:], zd[:], op=OP.add)

            nc.scalar.dma_start(out=hnew_d.ap(), in_=hnew_sb[:])
            # h/WOUT_SCALE in bf16: compensates the *WOUT_SCALE stored in the
            # fp8 projection weights (keeps them in e4m3's normal range)
            hs_f = small.tile([128, NJ], F32)
            nc.vector.tensor_scalar_mul(hs_f[:], hnew_sb[:], 1.0 / WOUT_SCALE)
            hcol_bf = small.tile([128, NJ], BF16)
            nc.vector.tensor_copy(hcol_bf[:], hs_f[:])

            # ---- output projection: logits + exp-sums, group by group ----
            logits_sb = small.tile([128, NT], F32)
            sums = small.tile([128, NG], F32)
            for g in range(NG):
                ps = psump.tile([128, GT], F32, tag="lps", bufs=2)
                for t in range(GT):
                    tg = g * GT + t          # global v-tile index
                    dg, td = divmod(tg, DT)  # DMA half + tile within it
                    for j in range(NJ):
                        nc.tensor.matmul(
                            out=ps[:, t : t + 1],
                            lhsT=w_sb[dg][j][:, td * 128 : (td + 1) * 128],
                            rhs=hcol_bf[:, j : j + 1],
                            start=(j == 0),
                            stop=(j == NJ - 1),
                        )
                gsl = slice(g * GT, (g + 1) * GT)
                nc.vector.tensor_tensor(
                    logits_sb[:, gsl], ps[:], bout_sb[:, gsl], op=OP.add
                )
                esc = small.tile([128, GT], F32, tag="esc", bufs=2)
                nc.scalar.activation(
                    esc[:], logits_sb[:, gsl], AF.Exp, accum_out=sums[:, g : g + 1]
                )

            # ---- global logsumexp via AllGather of per-core exp sums ----
            stot = small.tile([128, 1], F32)
            nc.vector.tensor_reduce(stot[:], sums[:], axis=mybir.AxisListType.X, op=OP.add)
            ps_s = psump.tile([1, 1], F32, tag="ps_s")
            nc.tensor.matmul(out=ps_s[:], lhsT=ones_sb[:], rhs=stot[:], start=True, stop=True)
            s_sb = small.tile([1, 1], F32)
            nc.scalar.copy(s_sb[:], ps_s[:])
            # replicate the scalar across 8 lanes so both collective bounce
            # buffers are single-descriptor contiguous transfers; the 8x
            # over-count is folded into the Ln scale below.
            s8 = small.tile([1, 8], F32)
            nc.vector.tensor_scalar(s8[:], zero8[:], s_sb[0:1, 0:1], None, OP.add)
            cc2_in = dram.tile([1, 8], F32)
            cc2_out = dram.tile([N_CORES, 8], F32, addr_space="Shared")
            nc.scalar.dma_start(out=cc2_in[:], in_=s8[:])
            # load the natural_log table set while AG is in flight
            duml = small.tile([1, 1], F32)
            nc.scalar.activation(duml[:], dum1[:], AF.Ln)
            nc.gpsimd.collective_compute(
                "AllGather",
                OP.bypass,
                replica_groups=[list(range(N_CORES))],
                ins=[cc2_in[:].opt()],
                outs=[cc2_out[:].opt()],
            )
            s64 = small.tile([1, 64], F32)
            nc.scalar.dma_start(
                out=s64[:], in_=cc2_out[:].rearrange("(o a) b -> o (a b)", o=1)
            )
            stot2 = small.tile([1, 1], F32)
            nc.vector.tensor_reduce(
                stot2[:], s64[:], axis=mybir.AxisListType.X, op=OP.add
            )
            # lse = ln(sum_c s_c) = ln(0.125 * sum(s64))
            lse1 = small.tile([1, 1], F32)
            nc.scalar.activation(lse1[:], stot2[:], AF.Ln, scale=0.125)
            # broadcast lse to all partitions via PE (ones_row.T @ lse1)
            ps_l = psump.tile([128, 1], F32, tag="ps_l")
            nc.tensor.matmul(out=ps_l[:], lhsT=ones_row[:], rhs=lse1[:], start=True, stop=True)
            lse_b = small.tile([128, 1], F32)
            nc.scalar.copy(lse_b[:], ps_l[:])

            logp_sb = small.tile([128, NT], F32)
            nc.vector.tensor_scalar(
                logp_sb[:], logits_sb[:], lse_b[:], None, OP.subtract
            )
            nc.scalar.dma_start(out=logp_d.ap(), in_=logp_sb[:])

    nc.compile()
    return nc


def get_nc():
    if "nc" not in _CACHE:
        _CACHE["nc"] = _build_nc()
    return _CACHE["nc"]


def prepare_in_maps(token, hidden, emb, w_ih, w_hh, b_ih, b_hh, w_out, b_out):
    token = np.asarray(token).reshape(-1)
    hidden = np.asarray(hidden, dtype=np.float32).reshape(HIDDEN)
    emb = np.asarray(emb, dtype=np.float32)
    w_ih = np.asarray(w_ih, dtype=np.float32)
    w_hh = np.asarray(w_hh, dtype=np.float32)
    b_ih = np.asarray(b_ih, dtype=np.float32)
    b_hh = np.asarray(b_hh, dtype=np.float32)
    w_out = np.asarray(w_out, dtype=np.float32)
    b_out = np.asarray(b_out, dtype=np.float32)

    emb_v = np.ascontiguousarray(emb).reshape(VOCAB * 128, 8).astype(BF16_NP)
    token32 = np.array([[int(token[0])]], dtype=np.int32)
    iota128 = np.arange(128, dtype=np.float32).reshape(128, 1)
    hid_col = np.ascontiguousarray(hidden.reshape(128, NJ))

    # GRU weights: [3h, h] -> [3, p, j, k, m]; element = W[g*1024+8m+j, 8p+k]
    def prep_gru(w, scale, np_dt):
        a = w.reshape(3, 128, NJ, 128, NJ)  # (g, m, j, p, k)
        return (np.ascontiguousarray(a.transpose(0, 3, 2, 4, 1)) * scale).astype(np_dt)

    wih_p = prep_gru(w_ih, GRU_SCALE, mybir.dt.np(FP8))
    whh_all = prep_gru(w_hh, 1.0, BF16_NP)          # [3, p, j, k, m] bf16
    whh_r = (whh_all[0:1].astype(np.float32) * GRU_SCALE).astype(mybir.dt.np(FP8))
    whh_zn = np.ascontiguousarray(whh_all[1:3])

    # biases in the interleaved [p, j] layout (value for gate row 8p+j)
    def il(v):
        return v.reshape(128, NJ)

    # full bias here: the 0.5 of the tanh-based sigmoid is applied by the ACT
    # scale AFTER the DVE bias-add (tanh(0.5*(psum + b)))
    b_r = il(b_ih[0:HIDDEN] + b_hh[0:HIDDEN])
    b_z = il(b_ih[HIDDEN : 2 * HIDDEN] + b_hh[HIDDEN : 2 * HIDDEN])
    b_in = il(b_ih[2 * HIDDEN :])
    b_hn = il(b_hh[2 * HIDDEN :])
    gru_bias = np.ascontiguousarray(
        np.concatenate([b_r, b_z, b_in, b_hn], axis=1)
    )

    # output projection: pad vocab, [V, h] -> per core [j, p, v] (h-col = 8p+j)
    w_out_pad = np.zeros((VPAD, HIDDEN), dtype=np.float32)
    w_out_pad[:VOCAB] = w_out
    wp = w_out_pad.reshape(N_CORES, VSH, 128, NJ)  # (c, v, p, j)
    b_out_pad = np.full(VPAD, PAD_BIAS, dtype=np.float32)
    b_out_pad[:VOCAB] = b_out
    bp = b_out_pad.reshape(N_CORES, NT, 128)

    in_maps = []
    for c in range(N_CORES):
        in_maps.append(
            {
                "emb_v": emb_v,
                "token32": token32,
                "iota128": iota128,
                "hid_col": hid_col,
                "gru_bias": gru_bias,
                "w_ih_p": wih_p,
                "w_hh_r": whh_r,
                "w_hh_p": whh_zn,
                "w_out_p": (
                    np.ascontiguousarray(wp[c].transpose(2, 1, 0)) * WOUT_SCALE
                ).astype(mybir.dt.np(FP8)),
                "b_out_col": np.ascontiguousarray(bp[c].T),
            }
        )
    return in_maps


def assemble_outputs(results):
    logp_pad = np.empty(VPAD, dtype=np.float32)
    for c in range(N_CORES):
        lp = results[c]["logp"]  # [128, NT], v_local = t*128 + p
        logp_pad[c * VSH : (c + 1) * VSH] = lp.T.reshape(VSH)
    # h_new is replicated; [128, 8] row-major == natural order (h = 8p+j)
    hnew = results[0]["h_new"].reshape(HIDDEN).astype(np.float32)
    return logp_pad[:VOCAB].reshape(1, VOCAB), hnew.reshape(1, 1, HIDDEN)


def run(inputs, **spmd_kwargs):
    nc = get_nc()
    in_maps = prepare_in_maps(**inputs)
    res = run_bass_kernel_spmd(nc, in_maps, core_ids=list(range(N_CORES)), **spmd_kwargs)
    return assemble_outputs(res.results), res


def kernel(**inputs):
    outputs, _ = run(inputs)
    return outputs
